# revision 1
# baseline (speedup 1.0000x reference)
"""GCN edge-classifier kernel for Trainium2, 8 NeuronCores.

Math reduction: with NCLASS=2, softmax(logits)[e] = [sigmoid(d), 1-sigmoid(d)]
where d = du[col_e] + dw[row_e] + (bfc0-bfc1),
  du[v] = dinv[v]*(t_u[v] + a_u[v]) + b1@wu,   (wu = Wfc[:64,0]-Wfc[:64,1])
  a_u   = dinv * (x @ (W1 @ wu)),              (scalar per node)
  t_u[v]= sum_{edges e: col_e==v} a_u[row_e],
  dinv  = rsqrt(1 + indegree)
(and likewise *_w with ww = Wfc[64:,0]-Wfc[64:,1] on the row side).

Sharding: edges are sharded across the 8 cores by target (col) range of
12500 nodes, sorted by col, and packed into 128-node "windows" of fixed
slot count so all aggregation is window-static PE one-hot matmuls.
"""
import numpy as np

N = 100000
E = 1600000
NFEAT = 256
NSH = 12500           # nodes per core
NPAD = 12544          # 98 * 128
NW = 98               # 128-node windows per core
WS = 19               # columns (x128 slots) per window
NCOL = NW * WS        # 1862 columns per core
SLOTS = NCOL * 128    # 238336 slots per core
NFULL = NPAD * 8      # 100352
ZROW = NPAD * 7 + 12510   # a zeroed pad row (core 7 block) in translated ids

_compiled = None


def _build():
    import concourse.bass as bass
    import concourse.bacc as bacc
    import concourse.mybir as mybir
    from concourse.tile import TileContext, add_dep_helper
    from concourse.masks import make_identity

    AluOp = mybir.AluOpType
    Act = mybir.ActivationFunctionType
    f32 = mybir.dt.float32
    i32 = mybir.dt.int32

    nc = bacc.Bacc('TRN2', target_bir_lowering=False, debug=False, num_devices=8)

    # inputs
    xT = nc.dram_tensor('xT', [NFEAT, NPAD], f32, kind='ExternalInput')
    cwin = nc.dram_tensor('cwin', [128, NCOL], f32, kind='ExternalInput')   # window-local col, junk=999
    rt = nc.dram_tensor('rt', [128, NCOL], i32, kind='ExternalInput')       # translated row ids (pad=ZROW)
    W1 = nc.dram_tensor('W1', [NFEAT, 64], f32, kind='ExternalInput')
    Wfc = nc.dram_tensor('Wfc', [128, 2], f32, kind='ExternalInput')
    b1 = nc.dram_tensor('b1', [64, 1], f32, kind='ExternalInput')
    bfc = nc.dram_tensor('bfc', [1, 2], f32, kind='ExternalInput')
    out = nc.dram_tensor('out', [128, NCOL, 2], f32, kind='ExternalOutput')

    # internal DRAM
    A_loc = nc.dram_tensor('A_loc', [NPAD, 2], f32)
    A_full = nc.dram_tensor('A_full', [NFULL, 2], f32, addr_space='Shared')
    D_loc = nc.dram_tensor('D_loc', [NPAD, 2], f32)
    D_full = nc.dram_tensor('D_full', [NFULL, 2], f32, addr_space='Shared')

    with TileContext(nc) as tc:
        with tc.tile_pool(name='cst', bufs=1) as cst, \
             tc.tile_pool(name='ps', bufs=1, space='PSUM') as ps, \
             tc.tile_pool(name='psw', bufs=2, space='PSUM') as psw, \
             tc.tile_pool(name='big', bufs=1) as big, \
             tc.tile_pool(name='wrk', bufs=3) as wrk:

            ident = cst.tile([128, 128], f32)
            make_identity(nc, ident[:])

            # ---- constants: wuw [64,2], q chunks, cbc [128,2] ----
            wfct = cst.tile([128, 2], f32)
            nc.sync.dma_start(out=wfct[:], in_=Wfc[:, :])
            diff = cst.tile([128, 1], f32)
            nc.vector.tensor_tensor(out=diff[:], in0=wfct[:, 0:1], in1=wfct[:, 1:2], op=AluOp.subtract)
            wuw = cst.tile([64, 2], f32)
            nc.vector.tensor_copy(out=wuw[0:64, 0:1], in_=diff[0:64, 0:1])
            nc.sync.dma_start(out=wuw[0:64, 1:2], in_=diff[64:128, 0:1])

            # W1T [64, 256] via PE transpose
            w1a = cst.tile([128, 64], f32)
            w1b = cst.tile([128, 64], f32)
            nc.sync.dma_start(out=w1a[:], in_=W1[0:128, :])
            nc.sync.dma_start(out=w1b[:], in_=W1[128:256, :])
            w1t = cst.tile([64, 256], f32)
            pt = ps.tile([64, 128], f32, tag='cstp')
            nc.tensor.transpose(out=pt[:], in_=w1a[:], identity=ident[:])
            nc.vector.tensor_copy(out=w1t[:, 0:128], in_=pt[:])
            pt2 = ps.tile([64, 128], f32, tag='cstp')
            nc.tensor.transpose(out=pt2[:], in_=w1b[:], identity=ident[:])
            nc.vector.tensor_copy(out=w1t[:, 128:256], in_=pt2[:])

            # q = W1 @ wuw  -> q_lo/q_hi [128, 2]
            q_lo = cst.tile([128, 2], f32)
            q_hi = cst.tile([128, 2], f32)
            pq = ps.tile([128, 128], f32, tag='cstp')
            nc.tensor.matmul(out=pq[:, 0:2], lhsT=w1t[:, 0:128], rhs=wuw[:], start=True, stop=True)
            nc.vector.tensor_copy(out=q_lo[:], in_=pq[:, 0:2])
            pq2 = ps.tile([128, 128], f32, tag='cstp')
            nc.tensor.matmul(out=pq2[:, 0:2], lhsT=w1t[:, 128:256], rhs=wuw[:], start=True, stop=True)
            nc.vector.tensor_copy(out=q_hi[:], in_=pq2[:, 0:2])

            # beta = b1 @ wuw [1,2]; db = bfc0-bfc1; cuw = beta + [db, 0]
            b1t = cst.tile([64, 1], f32)
            nc.sync.dma_start(out=b1t[:], in_=b1[:, :])
            pb = ps.tile([128, 128], f32, tag='cstp')
            nc.tensor.matmul(out=pb[0:1, 0:2], lhsT=b1t[:], rhs=wuw[:], start=True, stop=True)
            bfct = cst.tile([1, 2], f32)
            nc.sync.dma_start(out=bfct[:], in_=bfc[:, :])
            cuw1 = cst.tile([1, 2], f32)
            nc.vector.tensor_copy(out=cuw1[:], in_=pb[0:1, 0:2])
            dbt = cst.tile([1, 1], f32)
            nc.vector.tensor_tensor(out=dbt[:], in0=bfct[0:1, 0:1], in1=bfct[0:1, 1:2], op=AluOp.subtract)
            nc.vector.tensor_tensor(out=cuw1[0:1, 0:1], in0=cuw1[0:1, 0:1], in1=dbt[:], op=AluOp.add)
            # broadcast to [128, 2] via ones matmul
            ones1 = cst.tile([1, 128], f32)
            nc.vector.memset(ones1[:], 1.0)
            pcb = ps.tile([128, 128], f32, tag='cstp')
            nc.tensor.matmul(out=pcb[:, 0:2], lhsT=ones1[:], rhs=cuw1[:], start=True, stop=True)
            cbc = cst.tile([128, 2], f32)
            nc.vector.tensor_copy(out=cbc[:], in_=pcb[:, 0:2])

            # iota row [128,128] f32: value = free index
            iotai = cst.tile([128, 128], i32)
            nc.gpsimd.iota(iotai[:], pattern=[[1, 128]], base=0, channel_multiplier=0)
            iotaf = cst.tile([128, 128], f32)
            nc.vector.tensor_copy(out=iotaf[:], in_=iotai[:])

            ones128 = cst.tile([128, 1], f32)
            nc.vector.memset(ones128[:], 1.0)

            # ---- xq matvec: v-minor tiles ----
            xlo = big.tile([128, NPAD], f32, tag='xlo')
            xhi = big.tile([128, NPAD], f32, tag='xhi')
            nc.sync.dma_start(out=xlo[:], in_=xT[0:128, :])
            nc.sync.dma_start(out=xhi[:], in_=xT[128:256, :])
            xq = big.tile([128, NW, 2], f32, tag='xq')
            for g in range(NW):
                pxq = psw.tile([128, 2], f32, tag='acc')
                nc.tensor.matmul(out=pxq[:], lhsT=xlo[:, 128 * g:128 * (g + 1)], rhs=q_lo[:], start=True, stop=False)
                nc.tensor.matmul(out=pxq[:], lhsT=xhi[:, 128 * g:128 * (g + 1)], rhs=q_hi[:], start=False, stop=True)
                nc.vector.tensor_copy(out=xq[:, g, :], in_=pxq[:])

            # ---- load edge streams ----
            cw_sb = big.tile([128, NCOL], f32, tag='cw')
            rt_sb = big.tile([128, NCOL], i32, tag='rt')
            nc.sync.dma_start(out=cw_sb[:], in_=cwin[:, :])
            nc.sync.dma_start(out=rt_sb[:], in_=rt[:, :])

            # ---- deg pass: windowed one-hot matmuls ----
            deg = big.tile([128, NW], f32, tag='deg')
            for g in range(NW):
                pdeg = psw.tile([128, 2], f32, tag='acc')
                for j in range(WS):
                    col = g * WS + j
                    oh = wrk.tile([128, 128], f32, tag='oh')
                    nc.vector.tensor_tensor(
                        out=oh[:], in0=cw_sb[:, col:col + 1].to_broadcast([128, 128]),
                        in1=iotaf[:], op=AluOp.is_equal)
                    nc.tensor.matmul(out=pdeg[:, 0:1], lhsT=oh[:], rhs=ones128[:],
                                     start=(j == 0), stop=(j == WS - 1))
                nc.vector.tensor_copy(out=deg[:, g:g + 1], in_=pdeg[:, 0:1])

            # ---- dinv, A table ----
            sq = wrk.tile([128, NW], f32, tag='sq')
            nc.scalar.activation(out=sq[:], in_=deg[:], func=Act.Sqrt, bias=1.0, scale=1.0)
            dinv = big.tile([128, NW], f32, tag='dinv')
            nc.vector.reciprocal(out=dinv[:], in_=sq[:])
            A_sb = big.tile([128, NW, 2], f32, tag='A')
            nc.vector.tensor_tensor(out=A_sb[:, :, 0], in0=xq[:, :, 0], in1=dinv[:], op=AluOp.mult)
            nc.vector.tensor_tensor(out=A_sb[:, :, 1], in0=xq[:, :, 1], in1=dinv[:], op=AluOp.mult)
            # pad nodes (>=12500) are already zero: xT pad cols are host-zeroed,
            # no edges touch them, so xq=0 and dinv=1 there.
            wA = nc.sync.dma_start(out=A_loc.rearrange('(f p) c -> p f c', p=128), in_=A_sb[:])
            cc1 = nc.gpsimd.collective_compute(
                'AllGather', AluOp.bypass, replica_groups=[list(range(8))],
                ins=[A_loc[:, :]], outs=[A_full[:, :]])
            add_dep_helper(cc1.ins, wA.ins, True, 'allgather after A write')

            # ---- t pass ----
            t_sb = big.tile([128, NW, 2], f32, tag='t')
            for g in range(NW):
                ptw = psw.tile([128, 2], f32, tag='acc')
                for j in range(WS):
                    col = g * WS + j
                    ap = wrk.tile([128, 2], f32, tag='ap')
                    gi = nc.gpsimd.indirect_dma_start(
                        out=ap[:], out_offset=None, in_=A_full[:, :],
                        in_offset=bass.IndirectOffsetOnAxis(ap=rt_sb[:, col:col + 1], axis=0))
                    add_dep_helper(gi.ins, cc1.ins, True, 'gather after allgather')
                    oh = wrk.tile([128, 128], f32, tag='oh')
                    nc.vector.tensor_tensor(
                        out=oh[:], in0=cw_sb[:, col:col + 1].to_broadcast([128, 128]),
                        in1=iotaf[:], op=AluOp.is_equal)
                    nc.tensor.matmul(out=ptw[:], lhsT=oh[:], rhs=ap[:],
                                     start=(j == 0), stop=(j == WS - 1))
                nc.vector.tensor_copy(out=t_sb[:, g, :], in_=ptw[:])

            # ---- D tables ----
            D_sb = big.tile([128, NW, 2], f32, tag='D')
            tmp = wrk.tile([128, NW], f32, tag='tmp')
            nc.vector.tensor_tensor(out=tmp[:], in0=t_sb[:, :, 0], in1=A_sb[:, :, 0], op=AluOp.add)
            nc.vector.tensor_tensor(out=tmp[:], in0=tmp[:], in1=dinv[:], op=AluOp.mult)
            nc.vector.tensor_scalar(out=D_sb[:, :, 0], in0=tmp[:], scalar1=cbc[:, 0:1], scalar2=None, op0=AluOp.add)
            tmp2 = wrk.tile([128, NW], f32, tag='tmp2')
            nc.vector.tensor_tensor(out=tmp2[:], in0=t_sb[:, :, 1], in1=A_sb[:, :, 1], op=AluOp.add)
            nc.vector.tensor_tensor(out=tmp2[:], in0=tmp2[:], in1=dinv[:], op=AluOp.mult)
            nc.vector.tensor_scalar(out=D_sb[:, :, 1], in0=tmp2[:], scalar1=cbc[:, 1:2], scalar2=None, op0=AluOp.add)
            wD = nc.sync.dma_start(out=D_loc.rearrange('(f p) c -> p f c', p=128), in_=D_sb[:])
            cc2 = nc.gpsimd.collective_compute(
                'AllGather', AluOp.bypass, replica_groups=[list(range(8))],
                ins=[D_loc[:, :]], outs=[D_full[:, :]])
            add_dep_helper(cc2.ins, wD.ins, True, 'allgather after D write')

            # ---- output pass ----
            for g in range(NW):
                ow = wrk.tile([128, WS, 2], f32, tag='ow')
                for j in range(WS):
                    col = g * WS + j
                    dp = wrk.tile([128, 2], f32, tag='dp')
                    gi = nc.gpsimd.indirect_dma_start(
                        out=dp[:], out_offset=None, in_=D_full[:, :],
                        in_offset=bass.IndirectOffsetOnAxis(ap=rt_sb[:, col:col + 1], axis=0))
                    add_dep_helper(gi.ins, cc2.ins, True, 'gather after allgather2')
                    # gu via transposed one-hot: psum_oht = transpose(onehot)
                    oh = wrk.tile([128, 128], f32, tag='oh')
                    nc.vector.tensor_tensor(
                        out=oh[:], in0=cw_sb[:, col:col + 1].to_broadcast([128, 128]),
                        in1=iotaf[:], op=AluOp.is_equal)
                    poht = psw.tile([128, 128], f32, tag='poht')
                    nc.tensor.transpose(out=poht[:], in_=oh[:], identity=ident[:])
                    oht = wrk.tile([128, 128], f32, tag='oht')
                    nc.vector.tensor_copy(out=oht[:], in_=poht[:])
                    pgu = psw.tile([128, 2], f32, tag='acc2')
                    nc.tensor.matmul(out=pgu[:, 0:1], lhsT=oht[:], rhs=D_sb[:, g, 0:1], start=True, stop=True)
                    # delta = gu + dw[row]
                    delta = wrk.tile([128, 1], f32, tag='delta')
                    nc.vector.tensor_tensor(out=delta[:], in0=pgu[:, 0:1], in1=dp[:, 1:2], op=AluOp.add)
                    nc.scalar.activation(out=ow[:, j, 0:1], in_=delta[:], func=Act.Sigmoid, scale=1.0)
                    nc.scalar.activation(out=ow[:, j, 1:2], in_=delta[:], func=Act.Sigmoid, scale=-1.0)
                nc.sync.dma_start(out=out[:, g * WS:(g + 1) * WS, :], in_=ow[:])

    nc.compile()
    return nc


def _pack(x, edge_index, W1, b1, Wfc, bfc):
    c = np.asarray(edge_index[1], dtype=np.int64)
    r = np.asarray(edge_index[0], dtype=np.int64)
    order = np.argsort(c, kind='stable')
    sc = c[order]
    sr = r[order]
    spos = order

    in_maps = []
    unpack = []   # (core, col, partition) -> original edge pos
    for k in range(8):
        lo, hi = np.searchsorted(sc, [k * NSH, (k + 1) * NSH])
        ck = sc[lo:hi] - k * NSH          # local col in [0, 12500)
        rk = sr[lo:hi]
        pk = spos[lo:hi]
        # window-local packing
        cw = np.full((128, NCOL), 999.0, dtype=np.float32)
        rtr = np.full((128, NCOL), ZROW, dtype=np.int32)
        posmap = np.full((128, NCOL), -1, dtype=np.int64)
        win = ck // 128
        # slot edges of window g into columns [g*WS, (g+1)*WS)
        wlo = np.searchsorted(win, np.arange(NW))
        whi = np.searchsorted(win, np.arange(NW), side='right')
        maxcnt = (whi - wlo).max()
        assert maxcnt <= WS * 128, f'window overflow: {maxcnt}'
        for g in range(NW):
            a, b = wlo[g], whi[g]
            n = b - a
            if n == 0:
                continue
            i = np.arange(n)
            pp = i % 128
            jj = g * WS + i // 128
            cw[pp, jj] = (ck[a:b] - g * 128).astype(np.float32)
            rtr[pp, jj] = (NPAD * (rk[a:b] // NSH) + rk[a:b] % NSH).astype(np.int32)
            posmap[pp, jj] = pk[a:b]
        xk = np.zeros((NFEAT, NPAD), dtype=np.float32)
        xk[:, :NSH] = np.asarray(x[k * NSH:(k + 1) * NSH], dtype=np.float32).T
        in_maps.append({
            'xT': xk, 'cwin': cw, 'rt': rtr,
            'W1': np.asarray(W1, np.float32),
            'Wfc': np.asarray(Wfc, np.float32),
            'b1': np.asarray(b1, np.float32).reshape(64, 1),
            'bfc': np.asarray(bfc, np.float32).reshape(1, 2),
        })
        unpack.append(posmap)
    return in_maps, unpack


def kernel(x, edge_index, W1, b1, Wfc, bfc):
    global _compiled
    from concourse import bass_utils
    in_maps, unpack = _pack(x, edge_index, W1, b1, Wfc, bfc)
    if _compiled is None:
        _compiled = _build()
    res = bass_utils.run_bass_kernel_spmd(_compiled, in_maps, core_ids=list(range(8)))
    out = np.zeros((E, 2), dtype=np.float32)
    for k in range(8):
        o = res.results[k]['out']          # [128, NCOL, 2]
        pm = unpack[k]                     # [128, NCOL]
        mask = pm >= 0
        out[pm[mask]] = o[mask]
    return out



# revision 5
# speedup vs baseline: 3.5748x; 3.5748x over previous
"""GCN edge-classifier kernel for Trainium2, 8 NeuronCores — prefix-sum design.

Math reduction (NCLASS=2): softmax(logits)[e] = [sigmoid(d), 1-sigmoid(d)],
  d = du[col_e] + dw[row_e],
  du[v] = dinv[v]*(t_u[v] + a_u[v]) + b1@wu + (bfc0-bfc1),
  dw[v] = dinv[v]*(t_w[v] + a_w[v]) + b1@ww,
  a_*[v] = dinv[v]*(x[v]@q_*),  q_u = W1@wu, q_w = W1@ww,
  t_*[v] = sum over in-edges of a_*[row_e],  dinv = rsqrt(1+indeg).

Sharding: edges sharded by target (col) range of 12500 nodes per core, sorted
by col, packed densely p-major (edge i -> partition i//NCOL, column i%NCOL).
Aggregation t = segment sum over the col-sorted edge stream, computed as a
difference of inclusive prefix sums: per-partition scan along the free axis +
one strict-lower-triangular matmul for the cross-partition carry, then two
small boundary gathers per node. Per-edge a-values come from one batched
indirect gather out of the allgathered A table.
"""
import numpy as np

N = 100000
E = 1600000
NFEAT = 256
NSH = 12500            # nodes per core
NW = 98                # node windows of 128 (node v <-> [p=v%128, f=v//128])
NPAD = NW * 128        # 12544
NFULL = NPAD * 8       # 100352
NCOL = 1600            # dense edge columns per partition
SLOTS = 128 * NCOL     # 204800 edge slots per core
ZROW = NPAD * 7 + 12543  # a zeroed pad row (core 7 block) in translated ids
PROWS = 129 * NCOL     # P table rows: [0]=zero, [NCOL+i]=prefix through edge i

_compiled = None


def _build():
    import concourse.bass as bass
    import concourse.bacc as bacc
    import concourse.mybir as mybir
    from concourse.tile import TileContext, add_dep_helper

    AluOp = mybir.AluOpType
    Act = mybir.ActivationFunctionType
    f32 = mybir.dt.float32
    f16 = mybir.dt.float16
    i32 = mybir.dt.int32
    u16 = mybir.dt.uint16
    u8 = mybir.dt.uint8

    nc = bacc.Bacc('TRN2', target_bir_lowering=False, debug=False, num_devices=8)

    xT = nc.dram_tensor('xT', [NFEAT, NPAD], f16, kind='ExternalInput')
    q4 = nc.dram_tensor('q4', [128, 4], f16, kind='ExternalInput')
    cbc = nc.dram_tensor('cbc', [128, 2], f32, kind='ExternalInput')
    dinv2 = nc.dram_tensor('dinv2', [2, NPAD], f32, kind='ExternalInput')
    dinvn = nc.dram_tensor('dinvn', [128, NW * 2], f32, kind='ExternalInput')
    ct = nc.dram_tensor('ct', [128, NCOL], u16, kind='ExternalInput')
    rtlo = nc.dram_tensor('rtlo', [128, NCOL], u16, kind='ExternalInput')
    rthi = nc.dram_tensor('rthi', [128, NCOL], u8, kind='ExternalInput')
    e0 = nc.dram_tensor('e0', [128, NW], i32, kind='ExternalInput')
    e1 = nc.dram_tensor('e1', [128, NW], i32, kind='ExternalInput')
    out = nc.dram_tensor('out', [128, NCOL], f16, kind='ExternalOutput')

    A_loc = nc.dram_tensor('A_loc', [NPAD, 2], f32)
    A_full = nc.dram_tensor('A_full', [NFULL, 2], f32, addr_space='Shared')
    P_dram = nc.dram_tensor('P_dram', [PROWS, 2], f32)
    Du_loc = nc.dram_tensor('Du_loc', [NPAD, 1], f32)
    Dw_loc = nc.dram_tensor('Dw_loc', [NPAD, 1], f32)
    Dw_full = nc.dram_tensor('Dw_full', [NFULL, 1], f32, addr_space='Shared')

    with TileContext(nc) as tc:
        with tc.tile_pool(name='cst', bufs=1) as cst, \
             tc.tile_pool(name='big', bufs=1) as big, \
             tc.tile_pool(name='wrk', bufs=1) as wrk, \
             tc.tile_pool(name='ach', bufs=3) as ach, \
             tc.tile_pool(name='ps', bufs=2, space='PSUM') as ps, \
             tc.tile_pool(name='ps1', bufs=1, space='PSUM') as ps1:

            # ---- constants ----
            q_sb = cst.tile([128, 4], f16)
            nc.sync.dma_start(out=q_sb[:], in_=q4[:, :])
            cbc_sb = cst.tile([128, 2], f32)
            nc.sync.dma_start(out=cbc_sb[:], in_=cbc[:, :])
            dinvn_sb = cst.tile([128, NW, 2], f32)
            nc.sync.dma_start(out=dinvn_sb[:], in_=dinvn[:, :])
            e0_sb = cst.tile([128, NW], i32)
            nc.sync.dma_start(out=e0_sb[:], in_=e0[:, :])
            e1_sb = cst.tile([128, NW], i32)
            nc.sync.dma_start(out=e1_sb[:], in_=e1[:, :])
            dinv2_sb = cst.tile([2, NPAD], f32)
            nc.sync.dma_start(out=dinv2_sb[:], in_=dinv2[:, :])

            iotaP = cst.tile([128, 128], i32)
            nc.gpsimd.iota(iotaP[:], pattern=[[0, 128]], base=0, channel_multiplier=1)
            iotaF = cst.tile([128, 128], i32)
            nc.gpsimd.iota(iotaF[:], pattern=[[1, 128]], base=0, channel_multiplier=0)
            sltu = cst.tile([128, 128], f32)   # [k, m] = 1.0 if k < m
            nc.vector.tensor_tensor(out=sltu[:], in0=iotaP[:], in1=iotaF[:], op=AluOp.is_lt)

            # ---- edge streams ----
            ct_sb = big.tile([128, NCOL], u16)
            nc.sync.dma_start(out=ct_sb[:], in_=ct[:, :])
            rtlo_sb = big.tile([128, NCOL], u16)
            nc.sync.dma_start(out=rtlo_sb[:], in_=rtlo[:, :])
            rthi_sb = big.tile([128, NCOL], u8)
            nc.sync.dma_start(out=rthi_sb[:], in_=rthi[:, :])
            ct32 = big.tile([128, NCOL], i32)
            nc.vector.tensor_copy(out=ct32[:], in_=ct_sb[:])
            rtf = wrk.tile([128, NCOL], f32)
            nc.vector.tensor_copy(out=rtf[:], in_=rtlo_sb[:])
            rthf = wrk.tile([128, NCOL], f32)
            nc.vector.tensor_copy(out=rthf[:], in_=rthi_sb[:])
            nc.vector.tensor_scalar(out=rthf[:], in0=rthf[:], scalar1=65536.0,
                                    scalar2=None, op0=AluOp.mult)
            nc.vector.tensor_tensor(out=rtf[:], in0=rtf[:], in1=rthf[:], op=AluOp.add)
            rt32 = big.tile([128, NCOL], i32)
            nc.vector.tensor_copy(out=rt32[:], in_=rtf[:])

            # ---- xq matvec: a = dinv * (x @ q), layout [2, nodes] ----
            xlo = big.tile([128, NPAD], f16)
            nc.sync.dma_start(out=xlo[:], in_=xT[0:128, :])
            xhi = big.tile([128, NPAD], f16)
            nc.sync.dma_start(out=xhi[:], in_=xT[128:256, :])
            Aview = A_loc.rearrange('n c -> c n')   # [2, NPAD] view
            wAs = []
            for c0 in range(0, NPAD, 512):
                w = min(512, NPAD - c0)
                px = ps.tile([2, 512], f32, tag='px')
                nc.tensor.matmul(out=px[:, 0:w], lhsT=q_sb[:, 0:2],
                                 rhs=xlo[:, c0:c0 + w], start=True, stop=False)
                nc.tensor.matmul(out=px[:, 0:w], lhsT=q_sb[:, 2:4],
                                 rhs=xhi[:, c0:c0 + w], start=False, stop=True)
                ac = ach.tile([2, 512], f32, tag='ac')
                nc.vector.tensor_tensor(out=ac[:, 0:w], in0=px[:, 0:w],
                                        in1=dinv2_sb[:, c0:c0 + w], op=AluOp.mult)
                wAs.append(nc.sync.dma_start(out=Aview[:, c0:c0 + w], in_=ac[:, 0:w]))
            cc1 = nc.gpsimd.collective_compute(
                'AllGather', AluOp.bypass, replica_groups=[list(range(8))],
                ins=[A_loc[:, :]], outs=[A_full[:, :]])
            for wa in wAs:
                add_dep_helper(cc1.ins, wa.ins, True, 'allgather after A write')

            # ---- gather per-edge a values (multi-offset indirect DMA is
            # broken on HW; only [128, 1] offset columns are reliable) ----
            av = big.tile([128, NCOL, 2], f32)
            for j in range(NCOL):
                gi = nc.gpsimd.indirect_dma_start(
                    out=av[:, j, :], out_offset=None, in_=A_full[:, :],
                    in_offset=bass.IndirectOffsetOnAxis(ap=rt32[:, j:j + 1], axis=0))
                add_dep_helper(gi.ins, cc1.ins, True, 'gather after allgather')

            # ---- prefix sums: per-partition scan + cross-partition carry ----
            Pg = big.tile([128, NCOL, 2], f32)
            nc.vector.tensor_tensor_scan(
                out=Pg[:, :, 0], data0=av[:, :, 0], data1=av[:, :, 0],
                initial=0.0, op0=AluOp.add, op1=AluOp.bypass)
            nc.vector.tensor_tensor_scan(
                out=Pg[:, :, 1], data0=av[:, :, 1], data1=av[:, :, 1],
                initial=0.0, op0=AluOp.add, op1=AluOp.bypass)
            R = wrk.tile([128, 2], f32)
            nc.vector.tensor_copy(out=R[:], in_=Pg[:, NCOL - 1, :])
            pc = ps1.tile([128, 2], f32, tag='pc')
            nc.tensor.matmul(out=pc[:], lhsT=sltu[:], rhs=R[:], start=True, stop=True)
            carry = wrk.tile([128, 2], f32)
            nc.vector.tensor_copy(out=carry[:], in_=pc[:])
            nc.vector.tensor_tensor(out=Pg[:, :, 0], in0=Pg[:, :, 0],
                                    in1=carry[:, 0:1].to_broadcast([128, NCOL]), op=AluOp.add)
            nc.vector.tensor_tensor(out=Pg[:, :, 1], in0=Pg[:, :, 1],
                                    in1=carry[:, 1:2].to_broadcast([128, NCOL]), op=AluOp.add)

            z2 = cst.tile([1, 2], f32)
            nc.vector.memset(z2[:], 0.0)
            wz = nc.sync.dma_start(out=P_dram[0:1, :], in_=z2[:])
            Pview = P_dram.rearrange('(g j) c -> g j c', j=NCOL)  # [129, NCOL, 2]
            wP = nc.sync.dma_start(out=Pview[1:129], in_=Pg[:])

            # ---- boundary gathers -> t, D tables ----
            g1 = wrk.tile([128, NW, 2], f32)
            g0 = wrk.tile([128, NW, 2], f32)
            for f in range(NW):
                gi1 = nc.gpsimd.indirect_dma_start(
                    out=g1[:, f, :], out_offset=None, in_=P_dram[:, :],
                    in_offset=bass.IndirectOffsetOnAxis(ap=e1_sb[:, f:f + 1], axis=0))
                add_dep_helper(gi1.ins, wP.ins, True, 'boundary after P write')
                add_dep_helper(gi1.ins, wz.ins, True, 'boundary after P zero row')
                gi0 = nc.gpsimd.indirect_dma_start(
                    out=g0[:, f, :], out_offset=None, in_=P_dram[:, :],
                    in_offset=bass.IndirectOffsetOnAxis(ap=e0_sb[:, f:f + 1], axis=0))
                add_dep_helper(gi0.ins, wP.ins, True, 'boundary after P write')
                add_dep_helper(gi0.ins, wz.ins, True, 'boundary after P zero row')

            t_sb = wrk.tile([128, NW, 2], f32)
            nc.vector.tensor_tensor(out=t_sb[:], in0=g1[:], in1=g0[:], op=AluOp.subtract)
            a_n = wrk.tile([128, NW, 2], f32)
            wAn = nc.sync.dma_start(out=a_n[:], in_=A_loc.rearrange('(f p) c -> p f c', p=128))
            for wa in wAs:
                add_dep_helper(wAn.ins, wa.ins, True, 'A node view after A write')
            nc.vector.tensor_tensor(out=t_sb[:], in0=t_sb[:], in1=a_n[:], op=AluOp.add)
            nc.vector.tensor_tensor(out=t_sb[:], in0=t_sb[:], in1=dinvn_sb[:], op=AluOp.mult)
            Du_sb = wrk.tile([128, NW], f32)
            nc.vector.tensor_scalar(out=Du_sb[:], in0=t_sb[:, :, 0],
                                    scalar1=cbc_sb[:, 0:1], scalar2=None, op0=AluOp.add)
            Dw_sb = wrk.tile([128, NW], f32)
            nc.vector.tensor_scalar(out=Dw_sb[:], in0=t_sb[:, :, 1],
                                    scalar1=cbc_sb[:, 1:2], scalar2=None, op0=AluOp.add)
            wDu = nc.sync.dma_start(out=Du_loc.rearrange('(f p) c -> p f c', p=128),
                                    in_=Du_sb[:])
            wDw = nc.sync.dma_start(out=Dw_loc.rearrange('(f p) c -> p f c', p=128),
                                    in_=Dw_sb[:])
            cc2 = nc.gpsimd.collective_compute(
                'AllGather', AluOp.bypass, replica_groups=[list(range(8))],
                ins=[Dw_loc[:, :]], outs=[Dw_full[:, :]])
            add_dep_helper(cc2.ins, wDw.ins, True, 'allgather after Dw write')

            # ---- output pass ----
            dup = big.tile([128, NCOL], f32)
            dwp = big.tile([128, NCOL], f32)
            for j in range(NCOL):
                giu = nc.gpsimd.indirect_dma_start(
                    out=dup[:, j:j + 1], out_offset=None, in_=Du_loc[:, :],
                    in_offset=bass.IndirectOffsetOnAxis(ap=ct32[:, j:j + 1], axis=0))
                add_dep_helper(giu.ins, wDu.ins, True, 'du gather after Du write')
                giw = nc.gpsimd.indirect_dma_start(
                    out=dwp[:, j:j + 1], out_offset=None, in_=Dw_full[:, :],
                    in_offset=bass.IndirectOffsetOnAxis(ap=rt32[:, j:j + 1], axis=0))
                add_dep_helper(giw.ins, cc2.ins, True, 'dw gather after allgather2')
            nc.vector.tensor_tensor(out=dup[:], in0=dup[:], in1=dwp[:], op=AluOp.add)
            osb = big.tile([128, NCOL], f16)
            nc.scalar.activation(out=osb[:], in_=dup[:], func=Act.Sigmoid, scale=1.0)
            nc.sync.dma_start(out=out[:, :], in_=osb[:])

    nc.compile()
    return nc


def _pack(x, edge_index, W1, b1, Wfc, bfc):
    c = np.asarray(edge_index[1], dtype=np.int64)
    r = np.asarray(edge_index[0], dtype=np.int64)
    order = np.argsort(c, kind='stable')
    sc = c[order]
    sr = r[order]

    W1 = np.asarray(W1, np.float64)
    b1 = np.asarray(b1, np.float64)
    Wfc = np.asarray(Wfc, np.float64)
    bfc = np.asarray(bfc, np.float64)
    wu = Wfc[:64, 0] - Wfc[:64, 1]
    ww = Wfc[64:, 0] - Wfc[64:, 1]
    q = np.stack([W1 @ wu, W1 @ ww], axis=1).astype(np.float16)   # [256, 2]
    q4 = np.concatenate([q[:128], q[128:]], axis=1)               # [128, 4]
    cu = float(b1 @ wu + (bfc[0] - bfc[1]))
    cw = float(b1 @ ww)
    cbc = np.tile(np.array([[cu, cw]], np.float32), (128, 1))

    deg = (np.bincount(c, minlength=N) + 1).astype(np.float64)
    dinv_all = (1.0 / np.sqrt(deg)).astype(np.float32)

    x = np.asarray(x, np.float32)
    in_maps = []
    unpack = []
    for k in range(8):
        lo, hi = np.searchsorted(sc, [k * NSH, (k + 1) * NSH])
        nE = hi - lo
        assert nE <= SLOTS, f'core {k}: {nE} edges > {SLOTS} slots'
        ck = (sc[lo:hi] - k * NSH).astype(np.int64)   # local col, sorted
        rk = sr[lo:hi]
        pk = order[lo:hi]

        ctk = np.full(SLOTS, 12543, np.uint16)
        ctk[:nE] = ck
        rtr = np.full(SLOTS, ZROW, np.int64)
        rtr[:nE] = NPAD * (rk // NSH) + rk % NSH
        posmap = np.full(SLOTS, -1, np.int64)
        posmap[:nE] = pk

        # boundary ids into P_dram: row 0 is zero, row NCOL+i is prefix
        # through edge i; eid(s) = NCOL-1+s for s>=1 else 0
        s0 = np.searchsorted(ck, np.arange(NSH), side='left')
        s1 = np.searchsorted(ck, np.arange(NSH), side='right')
        eid0 = np.where(s0 > 0, NCOL - 1 + s0, 0).astype(np.int32)
        eid1 = np.where(s1 > 0, NCOL - 1 + s1, 0).astype(np.int32)
        e0k = np.zeros((128, NW), np.int32)
        e1k = np.zeros((128, NW), np.int32)
        v = np.arange(NSH)
        e0k[v % 128, v // 128] = eid0
        e1k[v % 128, v // 128] = eid1

        dl = dinv_all[k * NSH:(k + 1) * NSH]
        dinv2 = np.zeros((2, NPAD), np.float32)
        dinv2[:, :NSH] = dl
        dinvn = np.zeros((128, NW, 2), np.float32)
        dinvn[v % 128, v // 128, :] = dl[:, None]

        xk = np.zeros((NFEAT, NPAD), np.float16)
        xk[:, :NSH] = x[k * NSH:(k + 1) * NSH].T

        in_maps.append({
            'xT': xk, 'q4': q4.astype(np.float16), 'cbc': cbc,
            'dinv2': dinv2, 'dinvn': dinvn.reshape(128, NW * 2),
            'ct': ctk.reshape(128, NCOL),
            'rtlo': (rtr & 0xFFFF).astype(np.uint16).reshape(128, NCOL),
            'rthi': (rtr >> 16).astype(np.uint8).reshape(128, NCOL),
            'e0': e0k, 'e1': e1k,
        })
        unpack.append(posmap.reshape(128, NCOL))
    return in_maps, unpack


def kernel(x, edge_index, W1, b1, Wfc, bfc):
    global _compiled
    from concourse import bass_utils
    in_maps, unpack = _pack(x, edge_index, W1, b1, Wfc, bfc)
    if _compiled is None:
        _compiled = _build()
    res = bass_utils.run_bass_kernel_spmd(_compiled, in_maps, core_ids=list(range(8)))
    out = np.zeros((E, 2), dtype=np.float32)
    for k in range(8):
        o = np.asarray(res.results[k]['out'], np.float32)   # [128, NCOL]
        pm = unpack[k]
        mask = pm >= 0
        p0 = o[mask]
        out[pm[mask], 0] = p0
        out[pm[mask], 1] = 1.0 - p0
    return out


# revision 14
# speedup vs baseline: 4.7003x; 1.3148x over previous
"""GCN edge-classifier kernel for Trainium2, 8 NeuronCores — prefix-sum design.

Math reduction (NCLASS=2): softmax(logits)[e] = [sigmoid(d), 1-sigmoid(d)],
  d = du[col_e] + dw[row_e],
  du[v] = dinv[v]*(t_u[v] + a_u[v]) + b1@wu + (bfc0-bfc1),
  dw[v] = dinv[v]*(t_w[v] + a_w[v]) + b1@ww,
  a_*[v] = dinv[v]*(x[v]@q_*),  q_u = W1@wu, q_w = W1@ww,
  t_*[v] = sum over in-edges of a_*[row_e],  dinv = rsqrt(1+indeg).

Sharding: edges sharded by target (col) range of 12500 nodes per core, sorted
by col, packed densely p-major (edge i -> partition i//NCOL, column i%NCOL).
Aggregation t = segment sum over the col-sorted edge stream, computed as a
difference of inclusive prefix sums: per-partition scan along the free axis +
one strict-lower-triangular matmul for the cross-partition carry, then two
small boundary gathers per node. Per-edge a-values come from one batched
indirect gather out of the allgathered A table.
"""
import numpy as np

N = 100000
E = 1600000
NFEAT = 256
NSH = 12500            # nodes per core
NW = 98                # node windows of 128 (node v <-> [p=v%128, f=v//128])
NPAD = NW * 128        # 12544
NFULL = NPAD * 8       # 100352
NCOL = 1600            # dense edge columns per partition
SLOTS = 128 * NCOL     # 204800 edge slots per core
ZROW = NPAD * 7 + 12543  # a zeroed pad row (core 7 block) in translated ids
PROWS = 129 * NCOL     # P table rows: [0]=zero, [NCOL+i]=prefix through edge i

_compiled = None


def _build():
    import concourse.bass as bass
    import concourse.bacc as bacc
    import concourse.mybir as mybir
    from concourse.tile import TileContext, add_dep_helper

    AluOp = mybir.AluOpType
    Act = mybir.ActivationFunctionType
    f32 = mybir.dt.float32
    f16 = mybir.dt.float16
    i32 = mybir.dt.int32
    u16 = mybir.dt.uint16
    u8 = mybir.dt.uint8

    nc = bacc.Bacc('TRN2', target_bir_lowering=False, debug=False, num_devices=8)

    xT = nc.dram_tensor('xT', [NFEAT, NPAD], f16, kind='ExternalInput')
    q4 = nc.dram_tensor('q4', [128, 4], f16, kind='ExternalInput')
    cbc = nc.dram_tensor('cbc', [128, 2], f32, kind='ExternalInput')
    dinv2 = nc.dram_tensor('dinv2', [2, NPAD], f32, kind='ExternalInput')
    dinvn = nc.dram_tensor('dinvn', [128, NW * 2], f32, kind='ExternalInput')
    ct = nc.dram_tensor('ct', [128, NCOL], u16, kind='ExternalInput')
    rtlo = nc.dram_tensor('rtlo', [128, NCOL], u16, kind='ExternalInput')
    rthi = nc.dram_tensor('rthi', [128, NCOL], u8, kind='ExternalInput')
    e0 = nc.dram_tensor('e0', [128, NW], i32, kind='ExternalInput')
    e1 = nc.dram_tensor('e1', [128, NW], i32, kind='ExternalInput')
    sslot = nc.dram_tensor('sslot', [128, NW], i32, kind='ExternalInput')
    out = nc.dram_tensor('out', [128, NCOL], f16, kind='ExternalOutput')

    A_loc = nc.dram_tensor('A_loc', [NPAD, 2], f32)
    A_full = nc.dram_tensor('A_full', [NFULL, 2], f32, addr_space='Shared')
    P_dram = nc.dram_tensor('P_dram', [PROWS, 2], f32)
    Du_loc = nc.dram_tensor('Du_loc', [NPAD, 1], f32)
    Dw_loc = nc.dram_tensor('Dw_loc', [NPAD, 1], f32)
    Dw_full = nc.dram_tensor('Dw_full', [NFULL, 1], f32, addr_space='Shared')
    inj_dram = nc.dram_tensor('inj_dram', [PROWS, 1], f32)  # slot p*NCOL+j; dead rows >= SLOTS

    with TileContext(nc) as tc:
        with tc.tile_pool(name='cst', bufs=1) as cst, \
             tc.tile_pool(name='big', bufs=1) as big, \
             tc.tile_pool(name='wrk', bufs=1) as wrk, \
             tc.tile_pool(name='ach', bufs=3) as ach, \
             tc.tile_pool(name='ps', bufs=2, space='PSUM') as ps, \
             tc.tile_pool(name='ps1', bufs=1, space='PSUM') as ps1:

            # ---- constants ----
            q_sb = cst.tile([128, 4], f16)
            nc.sync.dma_start(out=q_sb[:], in_=q4[:, :])
            cbc_sb = cst.tile([128, 2], f32)
            nc.sync.dma_start(out=cbc_sb[:], in_=cbc[:, :])
            dinvn_sb = cst.tile([128, NW, 2], f32)
            nc.sync.dma_start(out=dinvn_sb[:], in_=dinvn[:, :])
            e0_sb = cst.tile([128, NW], i32)
            nc.sync.dma_start(out=e0_sb[:], in_=e0[:, :])
            e1_sb = cst.tile([128, NW], i32)
            nc.sync.dma_start(out=e1_sb[:], in_=e1[:, :])
            sslot_sb = cst.tile([128, NW], i32)
            nc.sync.dma_start(out=sslot_sb[:], in_=sslot[:, :])
            dinv2_sb = cst.tile([2, NPAD], f32)
            nc.sync.dma_start(out=dinv2_sb[:], in_=dinv2[:, :])

            iotaP = cst.tile([128, 128], i32)
            nc.gpsimd.iota(iotaP[:], pattern=[[0, 128]], base=0, channel_multiplier=1)
            iotaF = cst.tile([128, 128], i32)
            nc.gpsimd.iota(iotaF[:], pattern=[[1, 128]], base=0, channel_multiplier=0)
            sltu = cst.tile([128, 128], f32)   # [k, m] = 1.0 if k < m
            nc.vector.tensor_tensor(out=sltu[:], in0=iotaP[:], in1=iotaF[:], op=AluOp.is_lt)

            # ---- edge streams ----
            ct_sb = big.tile([128, NCOL], u16)
            nc.sync.dma_start(out=ct_sb[:], in_=ct[:, :])
            rtlo_sb = big.tile([128, NCOL], u16)
            nc.sync.dma_start(out=rtlo_sb[:], in_=rtlo[:, :])
            rthi_sb = big.tile([128, NCOL], u8)
            nc.sync.dma_start(out=rthi_sb[:], in_=rthi[:, :])
            ct32 = big.tile([128, NCOL], i32)
            nc.vector.tensor_copy(out=ct32[:], in_=ct_sb[:])
            rtf = wrk.tile([128, NCOL], f32)
            nc.vector.tensor_copy(out=rtf[:], in_=rtlo_sb[:])
            rthf = wrk.tile([128, NCOL], f32)
            nc.vector.tensor_copy(out=rthf[:], in_=rthi_sb[:])
            nc.vector.tensor_scalar(out=rthf[:], in0=rthf[:], scalar1=65536.0,
                                    scalar2=None, op0=AluOp.mult)
            nc.vector.tensor_tensor(out=rtf[:], in0=rtf[:], in1=rthf[:], op=AluOp.add)
            rt32 = big.tile([128, NCOL], i32)
            nc.vector.tensor_copy(out=rt32[:], in_=rtf[:])

            # ---- xq matvec: a = dinv * (x @ q), layout [2, nodes] ----
            xlo = big.tile([128, NPAD], f16)
            nc.sync.dma_start(out=xlo[:], in_=xT[0:128, :])
            xhi = big.tile([128, NPAD], f16)
            nc.sync.dma_start(out=xhi[:], in_=xT[128:256, :])
            Aview = A_loc.rearrange('n c -> c n')   # [2, NPAD] view
            wAs = []
            for c0 in range(0, NPAD, 512):
                w = min(512, NPAD - c0)
                px = ps.tile([2, 512], f32, tag='px')
                nc.tensor.matmul(out=px[:, 0:w], lhsT=q_sb[:, 0:2],
                                 rhs=xlo[:, c0:c0 + w], start=True, stop=False)
                nc.tensor.matmul(out=px[:, 0:w], lhsT=q_sb[:, 2:4],
                                 rhs=xhi[:, c0:c0 + w], start=False, stop=True)
                ac = ach.tile([2, 512], f32, tag='ac')
                nc.vector.tensor_tensor(out=ac[:, 0:w], in0=px[:, 0:w],
                                        in1=dinv2_sb[:, c0:c0 + w], op=AluOp.mult)
                wAs.append(nc.sync.dma_start(out=Aview[:, c0:c0 + w], in_=ac[:, 0:w]))
            cc1 = nc.gpsimd.collective_compute(
                'AllGather', AluOp.bypass, replica_groups=[list(range(8))],
                ins=[A_loc[:, :]], outs=[A_full[:, :]])
            for wa in wAs:
                add_dep_helper(cc1.ins, wa.ins, True, 'allgather after A write')

            # ---- gather per-edge a values (multi-offset indirect DMA is
            # broken on HW; only [128, 1] offset columns are reliable) ----
            av = big.tile([128, NCOL, 2], f32)
            for j in range(NCOL):
                gi = nc.gpsimd.indirect_dma_start(
                    out=av[:, j, :], out_offset=None, in_=A_full[:, :],
                    in_offset=bass.IndirectOffsetOnAxis(ap=rt32[:, j:j + 1], axis=0))
                add_dep_helper(gi.ins, cc1.ins, True, 'gather after allgather')

            # ---- prefix sums: per-partition scan + cross-partition carry ----
            Pg = big.tile([128, NCOL, 2], f32)
            nc.vector.tensor_tensor_scan(
                out=Pg[:, :, 0], data0=av[:, :, 0], data1=av[:, :, 0],
                initial=0.0, op0=AluOp.add, op1=AluOp.bypass)
            nc.vector.tensor_tensor_scan(
                out=Pg[:, :, 1], data0=av[:, :, 1], data1=av[:, :, 1],
                initial=0.0, op0=AluOp.add, op1=AluOp.bypass)
            R = wrk.tile([128, 2], f32)
            nc.vector.tensor_copy(out=R[:], in_=Pg[:, NCOL - 1, :])
            pc = ps1.tile([128, 2], f32, tag='pc')
            nc.tensor.matmul(out=pc[:], lhsT=sltu[:], rhs=R[:], start=True, stop=True)
            carry = wrk.tile([128, 2], f32)
            nc.vector.tensor_copy(out=carry[:], in_=pc[:])
            nc.vector.tensor_tensor(out=Pg[:, :, 0], in0=Pg[:, :, 0],
                                    in1=carry[:, 0:1].to_broadcast([128, NCOL]), op=AluOp.add)
            nc.vector.tensor_tensor(out=Pg[:, :, 1], in0=Pg[:, :, 1],
                                    in1=carry[:, 1:2].to_broadcast([128, NCOL]), op=AluOp.add)

            z2 = cst.tile([1, 2], f32)
            nc.vector.memset(z2[:], 0.0)
            wz = nc.sync.dma_start(out=P_dram[0:1, :], in_=z2[:])
            Pview = P_dram.rearrange('(g j) c -> g j c', j=NCOL)  # [129, NCOL, 2]
            wP = nc.sync.dma_start(out=Pview[1:129], in_=Pg[:])

            # ---- boundary gathers -> t, D tables ----
            g1 = wrk.tile([128, NW, 2], f32)
            g0 = wrk.tile([128, NW, 2], f32)
            for f in range(NW):
                gi1 = nc.gpsimd.indirect_dma_start(
                    out=g1[:, f, :], out_offset=None, in_=P_dram[:, :],
                    in_offset=bass.IndirectOffsetOnAxis(ap=e1_sb[:, f:f + 1], axis=0))
                add_dep_helper(gi1.ins, wP.ins, True, 'boundary after P write')
                add_dep_helper(gi1.ins, wz.ins, True, 'boundary after P zero row')
                gi0 = nc.gpsimd.indirect_dma_start(
                    out=g0[:, f, :], out_offset=None, in_=P_dram[:, :],
                    in_offset=bass.IndirectOffsetOnAxis(ap=e0_sb[:, f:f + 1], axis=0))
                add_dep_helper(gi0.ins, wP.ins, True, 'boundary after P write')
                add_dep_helper(gi0.ins, wz.ins, True, 'boundary after P zero row')

            t_sb = wrk.tile([128, NW, 2], f32)
            nc.vector.tensor_tensor(out=t_sb[:], in0=g1[:], in1=g0[:], op=AluOp.subtract)
            a_n = wrk.tile([128, NW, 2], f32)
            wAn = nc.sync.dma_start(out=a_n[:], in_=A_loc.rearrange('(f p) c -> p f c', p=128))
            for wa in wAs:
                add_dep_helper(wAn.ins, wa.ins, True, 'A node view after A write')
            nc.vector.tensor_tensor(out=t_sb[:], in0=t_sb[:], in1=a_n[:], op=AluOp.add)
            nc.vector.tensor_tensor(out=t_sb[:], in0=t_sb[:], in1=dinvn_sb[:], op=AluOp.mult)
            Du_sb = wrk.tile([128, NW], f32)
            nc.vector.tensor_scalar(out=Du_sb[:], in0=t_sb[:, :, 0],
                                    scalar1=cbc_sb[:, 0:1], scalar2=None, op0=AluOp.add)
            Dw_sb = wrk.tile([128, NW], f32)
            nc.vector.tensor_scalar(out=Dw_sb[:], in0=t_sb[:, :, 1],
                                    scalar1=cbc_sb[:, 1:2], scalar2=None, op0=AluOp.add)
            wDu = nc.sync.dma_start(out=Du_loc.rearrange('(f p) c -> p f c', p=128),
                                    in_=Du_sb[:])
            wDw = nc.sync.dma_start(out=Dw_loc.rearrange('(f p) c -> p f c', p=128),
                                    in_=Dw_sb[:])
            cc2 = nc.gpsimd.collective_compute(
                'AllGather', AluOp.bypass, replica_groups=[list(range(8))],
                ins=[Dw_loc[:, :]], outs=[Dw_full[:, :]])
            add_dep_helper(cc2.ins, wDw.ins, True, 'allgather after Dw write')

            # ---- output pass ----
            # dup (= Du[ct]) is piecewise-constant along each partition's
            # col-sorted edge row: scatter Du to segment-start slots, then an
            # affine scan state = keep*state + inject expands it per slot.
            zNC = big.tile([128, NCOL], f32)
            nc.vector.memset(zNC[:], 0.0)
            Iview = inj_dram.rearrange('(p j) c -> p j c', p=129)  # [129, NCOL, 1]
            wzi = nc.sync.dma_start(out=Iview[0:128], in_=zNC[:])
            wss = []
            for f in range(NW):
                si = nc.gpsimd.indirect_dma_start(
                    out=inj_dram[:, :],
                    out_offset=bass.IndirectOffsetOnAxis(ap=sslot_sb[:, f:f + 1], axis=0),
                    in_=Du_sb[:, f:f + 1], in_offset=None)
                add_dep_helper(si.ins, wzi.ins, True, 'scatter after inj zero')
                wss.append(si)
            inj_sb = big.tile([128, NCOL], f32)
            winj = nc.sync.dma_start(out=inj_sb[:], in_=Iview[0:128, :, 0])
            for si in wss:
                add_dep_helper(winj.ins, si.ins, True, 'inj load after scatter')
            gic = nc.gpsimd.indirect_dma_start(
                out=inj_sb[:, 0:1], out_offset=None, in_=Du_loc[:, :],
                in_offset=bass.IndirectOffsetOnAxis(ap=ct32[:, 0:1], axis=0))
            add_dep_helper(gic.ins, wDu.ins, True, 'row-start Du gather')
            ke = big.tile([128, NCOL], f32)
            nc.vector.tensor_tensor(out=ke[:, 1:NCOL], in0=ct_sb[:, 1:NCOL],
                                    in1=ct_sb[:, 0:NCOL - 1], op=AluOp.is_equal)
            nc.vector.memset(ke[:, 0:1], 0.0)
            dup = big.tile([128, NCOL], f32)
            nc.vector.tensor_tensor_scan(
                out=dup[:], data0=ke[:], data1=inj_sb[:],
                initial=0.0, op0=AluOp.mult, op1=AluOp.add)

            dwp = big.tile([128, NCOL], f32)
            for j in range(NCOL):
                giw = nc.gpsimd.indirect_dma_start(
                    out=dwp[:, j:j + 1], out_offset=None, in_=Dw_full[:, :],
                    in_offset=bass.IndirectOffsetOnAxis(ap=rt32[:, j:j + 1], axis=0))
                add_dep_helper(giw.ins, cc2.ins, True, 'dw gather after allgather2')
            nc.vector.tensor_tensor(out=dup[:], in0=dup[:], in1=dwp[:], op=AluOp.add)
            osb = big.tile([128, NCOL], f16)
            nc.scalar.activation(out=osb[:], in_=dup[:], func=Act.Sigmoid, scale=1.0)
            nc.sync.dma_start(out=out[:, :], in_=osb[:])

    nc.compile()
    return nc


def _pack(x, edge_index, W1, b1, Wfc, bfc):
    c = np.asarray(edge_index[1], dtype=np.int64)
    r = np.asarray(edge_index[0], dtype=np.int64)
    order = np.argsort(c, kind='stable')
    sc = c[order]
    sr = r[order]

    W1 = np.asarray(W1, np.float64)
    b1 = np.asarray(b1, np.float64)
    Wfc = np.asarray(Wfc, np.float64)
    bfc = np.asarray(bfc, np.float64)
    wu = Wfc[:64, 0] - Wfc[:64, 1]
    ww = Wfc[64:, 0] - Wfc[64:, 1]
    q = np.stack([W1 @ wu, W1 @ ww], axis=1).astype(np.float16)   # [256, 2]
    q4 = np.concatenate([q[:128], q[128:]], axis=1)               # [128, 4]
    cu = float(b1 @ wu + (bfc[0] - bfc[1]))
    cw = float(b1 @ ww)
    cbc = np.tile(np.array([[cu, cw]], np.float32), (128, 1))

    deg = (np.bincount(c, minlength=N) + 1).astype(np.float64)
    dinv_all = (1.0 / np.sqrt(deg)).astype(np.float32)

    x = np.asarray(x, np.float32)
    in_maps = []
    unpack = []
    for k in range(8):
        lo, hi = np.searchsorted(sc, [k * NSH, (k + 1) * NSH])
        nE = hi - lo
        assert nE <= SLOTS, f'core {k}: {nE} edges > {SLOTS} slots'
        ck = (sc[lo:hi] - k * NSH).astype(np.int64)   # local col, sorted
        rk = sr[lo:hi]
        pk = order[lo:hi]

        ctk = np.full(SLOTS, 12543, np.uint16)
        ctk[:nE] = ck
        rtr = np.full(SLOTS, ZROW, np.int64)
        rtr[:nE] = NPAD * (rk // NSH) + rk % NSH
        posmap = np.full(SLOTS, -1, np.int64)
        posmap[:nE] = pk

        # boundary ids into P_dram: row 0 is zero, row NCOL+i is prefix
        # through edge i; eid(s) = NCOL-1+s for s>=1 else 0
        s0 = np.searchsorted(ck, np.arange(NSH), side='left')
        s1 = np.searchsorted(ck, np.arange(NSH), side='right')
        eid0 = np.where(s0 > 0, NCOL - 1 + s0, 0).astype(np.int32)
        eid1 = np.where(s1 > 0, NCOL - 1 + s1, 0).astype(np.int32)
        e0k = np.zeros((128, NW), np.int32)
        e1k = np.zeros((128, NW), np.int32)
        v = np.arange(NSH)
        e0k[v % 128, v // 128] = eid0
        e1k[v % 128, v // 128] = eid1
        # scatter target for Du injection: first slot of v's segment, or a
        # dead row (distinct per partition) for empty/pad nodes
        sslk = SLOTS + np.tile(np.arange(128, dtype=np.int32)[:, None], (1, NW))
        ssl = np.where(s1 > s0, s0, SLOTS + (v % 128)).astype(np.int32)
        sslk[v % 128, v // 128] = ssl

        dl = dinv_all[k * NSH:(k + 1) * NSH]
        dinv2 = np.zeros((2, NPAD), np.float32)
        dinv2[:, :NSH] = dl
        dinvn = np.zeros((128, NW, 2), np.float32)
        dinvn[v % 128, v // 128, :] = dl[:, None]

        xk = np.zeros((NFEAT, NPAD), np.float16)
        xk[:, :NSH] = x[k * NSH:(k + 1) * NSH].T

        in_maps.append({
            'xT': xk, 'q4': q4.astype(np.float16), 'cbc': cbc,
            'dinv2': dinv2, 'dinvn': dinvn.reshape(128, NW * 2),
            'ct': ctk.reshape(128, NCOL),
            'rtlo': (rtr & 0xFFFF).astype(np.uint16).reshape(128, NCOL),
            'rthi': (rtr >> 16).astype(np.uint8).reshape(128, NCOL),
            'e0': e0k, 'e1': e1k, 'sslot': sslk,
        })
        unpack.append(posmap.reshape(128, NCOL))
    return in_maps, unpack


def _enable_jax_cache():
    try:
        import jax
        jax.config.update('jax_compilation_cache_dir', '/tmp/jaxcache')
        jax.config.update('jax_persistent_cache_min_compile_time_secs', 0.0)
        jax.config.update('jax_persistent_cache_min_entry_size_bytes', 0)
    except Exception:
        pass


def kernel(x, edge_index, W1, b1, Wfc, bfc):
    global _compiled
    _enable_jax_cache()
    from concourse import bass_utils
    in_maps, unpack = _pack(x, edge_index, W1, b1, Wfc, bfc)
    if _compiled is None:
        _compiled = _build()
    res = bass_utils.run_bass_kernel_spmd(_compiled, in_maps, core_ids=list(range(8)))
    out = np.zeros((E, 2), dtype=np.float32)
    for k in range(8):
        o = np.asarray(res.results[k]['out'], np.float32)   # [128, NCOL]
        pm = unpack[k]
        mask = pm >= 0
        p0 = o[mask]
        out[pm[mask], 0] = p0
        out[pm[mask], 1] = 1.0 - p0
    return out
